# revision 3
# baseline (speedup 1.0000x reference)
"""Trainium2 Bass kernel v6: v5 + wave-split normalization.

Same GEMV/softmax design as v5 (see kernel.py docstring), plus: slots
whose columns complete early (wave A, all but the last ~24 columns) get
their entire normalization chain -- cross-partition sum, reciprocal,
broadcast, selection-matrix expand, multiply, and output DMA -- emitted
mid-stream where it hides under the remaining input DMA.  Only the few
last slots (wave B) run their (latency-bound) chain after the final
input group, so the exposed tail shrinks from ~6 us to ~3 us.
"""

import time

import numpy as np

EMBED = 512
LMAX = 2048
NCORES = 8
B2 = 256
SEQS = B2 // NCORES        # 32 sequences per core, one per column-slot
TILE = 128                 # tokens per PSUM column (= stationary cols)
GCOLS = 16                 # columns per DMA group / PSUM bank (2 MB fp16)
QBUFS = 4                  # input tile buffering depth
PSBUFS = 4                 # rotating PSUM banks for the GEMV
QDT = "f16"                # wire dtype: "f16" or "f8" (e4m3)
WAVE_TAIL = 24             # target wave-B width in columns

_nc_cache = {}


def _schedule(lens):
    """Sort rows by tile count, deal round-robin to cores; every core's
    slot j holds K[j] = max tile count of the 8 rows in that slot."""
    k = (lens + TILE - 1) // TILE            # [256] tiles per row (>=1)
    order = np.argsort(-k, kind="stable")
    K = np.empty(SEQS, np.int64)
    for j in range(SEQS):
        K[j] = k[order[j * NCORES:(j + 1) * NCORES]].max()
    B = np.zeros(SEQS, np.int64)
    B[1:] = np.cumsum(K)[:-1]
    T = int(B[-1] + K[-1])
    Tcols = ((T + 3) // 4) * 4
    assert Tcols <= 512, "one PSUM bank holds <= 512 fp32 columns"
    return order, K, B, Tcols


def _groups(Tcols):
    """Group sizes: GCOLS-wide, tapering to 4 at the end so the PE work
    exposed after the last DMA is small."""
    sizes = []
    rest = Tcols
    taper = [8, 4, 4]
    while rest > sum(taper) + GCOLS - 1:
        sizes.append(GCOLS)
        rest -= GCOLS
    while rest >= 4:
        for t in taper:
            if rest >= t + (4 if t > 4 else 0) or rest == t:
                sizes.append(t)
                rest -= t
                break
        else:
            sizes.append(4)
            rest -= 4
    assert rest == 0 and sum(sizes) == Tcols, (sizes, Tcols)
    return sizes


def _split(K, B, Tcols):
    """Wave split: jA = count of slots ending at or before Tcols-WAVE_TAIL;
    CA = first wave-B column.  Guarantees 1 <= jA < SEQS."""
    ends = B + K
    jA = int(np.searchsorted(ends, Tcols - WAVE_TAIL, side="right"))
    jA = max(1, min(SEQS - 1, jA))
    CA = int(B[jA])
    return jA, CA


def _build_nc(Tcols, K, B):
    from contextlib import ExitStack

    import concourse.bass as bass
    import concourse.tile as tile
    from concourse import bacc, mybir

    fq = mybir.dt.float8e4 if QDT == "f8" else mybir.dt.float16
    f32 = mybir.dt.float32
    Ttok = Tcols * TILE
    sizes = _groups(Tcols)
    starts = np.zeros(len(sizes), np.int64)
    starts[1:] = np.cumsum(sizes)[:-1]
    ngrp = len(sizes)
    jA, CA = _split(K, B, Tcols)
    nB = SEQS - jA

    # slot j's columns are complete after the group containing B[j]+K[j]-1
    reduces_after = {g: [] for g in range(ngrp)}
    for j in range(SEQS):
        last = int(B[j]) + int(K[j]) - 1
        g = int(np.searchsorted(starts, last, side="right")) - 1
        reduces_after[g].append(j)
    # emit wave-A chain two groups after the one exp'ing column CA-1
    gA = int(np.searchsorted(starts, CA - 1, side="right")) - 1
    gA_emit = min(gA + 2, ngrp - 1)

    nc = bacc.Bacc("TRN2", target_bir_lowering=False, debug=False,
                   num_devices=NCORES)
    q_h = nc.dram_tensor("qpt", [4, 128, Ttok], fq, kind="ExternalInput")
    w_h = nc.dram_tensor("w", [128, 4], fq, kind="ExternalInput")
    e_h = nc.dram_tensor("esel", [SEQS, Tcols], f32, kind="ExternalInput")
    eb_h = nc.dram_tensor("eselb", [SEQS, Tcols], f32, kind="ExternalInput")
    out_h = nc.dram_tensor("out", [128, Tcols], f32, kind="ExternalOutput")

    with tile.TileContext(nc) as tc, ExitStack() as ctx:
        singles = ctx.enter_context(tc.tile_pool(name="singles", bufs=1))
        qpool = ctx.enter_context(tc.tile_pool(name="qpool", bufs=QBUFS))
        psum = ctx.enter_context(tc.tile_pool(name="psum", bufs=PSBUFS,
                                              space="PSUM"))
        psum1 = ctx.enter_context(tc.tile_pool(name="psum1", bufs=1,
                                               space="PSUM"))

        w_sb = singles.tile([128, 4], fq)
        nc.sync.dma_start(out=w_sb, in_=w_h.ap())
        e_sb = singles.tile([SEQS, Tcols], f32)
        nc.sync.dma_start(out=e_sb, in_=e_h.ap())
        eb_sb = singles.tile([SEQS, Tcols], f32)
        nc.sync.dma_start(out=eb_sb, in_=eb_h.ap())
        ones_k = singles.tile([128, 1], f32)
        nc.vector.memset(ones_k, 1.0)
        ones_m = singles.tile([1, 128], f32)
        nc.vector.memset(ones_m, 1.0)

        expm = singles.tile([128, Tcols], f32)
        sums = singles.tile([128, SEQS], f32)
        rec = singles.tile([1, SEQS], f32)
        recbT = singles.tile([SEQS, 128], f32)
        recbTB = singles.tile([SEQS, 128], f32)
        outt = singles.tile([128, Tcols], f32)
        s_ps = psum1.tile([1, SEQS], f32, tag="s_ps")
        r2_ps = psum1.tile([SEQS, 128], f32, tag="r2_ps")
        r2b_ps = psum1.tile([SEQS, 128], f32, tag="r2b_ps")
        sc_ps = psum1.tile([128, Tcols], f32, tag="sc_ps")

        def chain_a():
            nc.tensor.matmul(s_ps[:, :jA], ones_k, sums[:, :jA],
                             start=True, stop=True)
            nc.vector.reciprocal(rec[:, :jA], s_ps[:, :jA])
            nc.tensor.matmul(r2_ps[:jA, :], rec[:, :jA], ones_m,
                             start=True, stop=True)
            nc.vector.tensor_copy(recbT[:jA, :], r2_ps[:jA, :])
            nc.tensor.matmul(sc_ps[:, :CA], recbT[:jA, :], e_sb[:jA, :CA],
                             start=True, stop=True)
            nc.vector.tensor_mul(outt[:, :CA], expm[:, :CA], sc_ps[:, :CA])
            nc.sync.dma_start(out=out_h.ap()[:, :CA], in_=outt[:, :CA])

        # ---- GEMV + pipelined exp/rowsum, wave-A chain mid-stream
        for g in range(ngrp):
            g0, gn = int(starts[g]), int(sizes[g])
            qt = qpool.tile([128, 4, gn * TILE], fq, tag=f"qt{gn}")
            nc.sync.dma_start(
                out=qt,
                in_=bass.AP(tensor=q_h, offset=g0 * TILE,
                            ap=[[Ttok, 128], [128 * Ttok, 4],
                                [1, gn * TILE]]))
            e_ps = psum.tile([128, 512], f32, tag="eps")
            for tt in range(gn):
                for c in range(4):
                    nc.tensor.matmul(e_ps[:, tt:tt + 1],
                                     qt[:, c, tt * TILE:(tt + 1) * TILE],
                                     w_sb[:, c:c + 1],
                                     start=(c == 0), stop=(c == 3))
            nc.scalar.activation(out=expm[:, g0:g0 + gn],
                                 in_=e_ps[:, :gn],
                                 func=mybir.ActivationFunctionType.Exp)
            for j in reduces_after[g]:
                nc.vector.tensor_reduce(out=sums[:, j:j + 1],
                                        in_=expm[:, B[j]:B[j] + K[j]],
                                        axis=mybir.AxisListType.X,
                                        op=mybir.AluOpType.add)
            if g == gA_emit:
                chain_a()
        if gA_emit >= ngrp:  # pragma: no cover (gA_emit clamped above)
            chain_a()

        # ---- wave B: remaining slots, columns [CA, Tcols)
        nc.tensor.matmul(s_ps[:, jA:], ones_k, sums[:, jA:],
                         start=True, stop=True)
        nc.vector.reciprocal(rec[:, jA:], s_ps[:, jA:])
        nc.tensor.matmul(r2b_ps[:nB, :], rec[:, jA:], ones_m,
                         start=True, stop=True)
        nc.vector.tensor_copy(recbTB[:nB, :], r2b_ps[:nB, :])
        nc.tensor.matmul(sc_ps[:, CA:], recbTB[:nB, :], eb_sb[:nB, CA:],
                         start=True, stop=True)
        nc.vector.tensor_mul(outt[:, CA:], expm[:, CA:], sc_ps[:, CA:])
        nc.sync.dma_start(out=out_h.ap()[:, CA:], in_=outt[:, CA:])

    nc.compile()
    return nc


def _get_nc(Tcols, K, B):
    key = (Tcols, tuple(int(x) for x in K))
    if key not in _nc_cache:
        _nc_cache[key] = _build_nc(Tcols, K, B)
    return _nc_cache[key]


def prepare(questions, questions_lens, lin_w, weight_vec):
    """Host-side sharding: schedule, fold W into u, pack/cast/transpose."""
    import ml_dtypes

    q = np.asarray(questions)
    lens = np.asarray(questions_lens).astype(np.int64).ravel()
    w = np.asarray(lin_w, dtype=np.float64)
    v = np.asarray(weight_vec, dtype=np.float64)
    u = (w.T @ v).astype(np.float32)

    order, K, B, Tcols = _schedule(lens)
    jA, CA = _split(K, B, Tcols)
    Ttok = Tcols * TILE
    npdt = ml_dtypes.float8_e4m3 if QDT == "f8" else np.float16
    unorm = float(u.astype(np.float64) @ u.astype(np.float64))
    pad_tok = ((-60.0 / unorm) * u).astype(npdt)   # energy ~ -60 -> exp ~ 0
    w_sb = np.ascontiguousarray(
        u.reshape(4, 128).T.astype(npdt))          # w_sb[p, c] = u[c*128+p]
    esel = np.zeros((SEQS, Tcols), np.float32)
    for j in range(SEQS):
        esel[j, B[j]:B[j] + K[j]] = 1.0
    eselb = np.zeros((SEQS, Tcols), np.float32)
    eselb[:SEQS - jA] = esel[jA:]                  # wave-B rows, shifted to 0

    in_maps = []
    for c in range(NCORES):
        buf = np.empty((Ttok, EMBED), npdt)
        buf[:] = pad_tok
        for j in range(SEQS):
            r = order[j * NCORES + c]
            n = int(lens[r])
            buf[B[j] * TILE:B[j] * TILE + n] = q[r, :n]
        qpt = np.ascontiguousarray(buf.T).reshape(4, 128, Ttok)
        in_maps.append({"qpt": qpt, "w": w_sb, "esel": esel, "eselb": eselb})
    return in_maps, (order, K, B, Tcols, lens)


def unpack(core_outs, meta):
    order, K, B, Tcols, lens = meta
    full = np.zeros((B2, LMAX), np.float32)
    for c in range(NCORES):
        o = np.asarray(core_outs[c])                 # [128, Tcols]
        for j in range(SEQS):
            r = order[j * NCORES + c]
            n = int(lens[r])
            blk = o[:, B[j]:B[j] + K[j]]             # [128 tok, K_j tiles]
            full[r, :n] = blk.T.reshape(-1)[:n]
    return full


def run_sharded(questions, questions_lens, lin_w, lin_b, weight_vec,
                trace=False):
    """Shard across the 8 cores, run, gather.  Returns (out, results)."""
    from concourse.bass_utils import run_bass_kernel_spmd

    in_maps, meta = prepare(questions, questions_lens, lin_w, weight_vec)
    nc = _get_nc(meta[3], meta[1], meta[2])

    res = None
    last_err = None
    for attempt in range(5):
        try:
            res = run_bass_kernel_spmd(nc, in_maps,
                                       core_ids=list(range(NCORES)),
                                       trace=trace)
            break
        except ModuleNotFoundError:
            trace = False
            continue
        except Exception as e:  # device left unrecoverable by a prior crash
            last_err = e
            if "UNAVAILABLE" in str(e) or "UNRECOVERABLE" in str(e):
                time.sleep(20 * (attempt + 1))
                continue
            raise
    if res is None:
        raise last_err
    out = unpack([r["out"] for r in res.results], meta)
    return out, res


def kernel(questions, questions_lens, lin_w, lin_b, weight_vec):
    out, _ = run_sharded(questions, questions_lens, lin_w, lin_b, weight_vec)
    return out
